# revision 43
# baseline (speedup 1.0000x reference)
"""MiniMax-M2 MoE kernel for 8 Trainium2 NeuronCores.

Strategy (expert-parallel, sparse/routed):
  Host: router gate matmul + sigmoid + top-4 selection + combine-weight
    renormalization in f32 numpy (pure data movement / tiny matmul), then
    gather tokens per expert, pad each expert slot to a static capacity.
  Host: quantize activations and weights to fp8-e4m3 hi/lo pairs
    (value = hi + lo exactly to ~2^-14 relative) so the device can run all
    matmuls in DoubleRow fp8 perf mode while keeping ~bf16 accuracy via
    3-product error compensation (hi*hi + hi*lo + lo*hi; lo*lo dropped).
  Device (expert-parallel): per core, 2-3 expert slots' SwiGLU FFN over
    the gathered tokens (slot plan chosen to minimize the static per-core
    token capacity; heavy experts may spill a small slice into a third
    slot on another core); combine weight applied on device; bf16 outputs.
  Host: scatter-add per-expert outputs into the [T, H] result in expert
    order (matches the reference scan accumulation order).

Scale bookkeeping (per-tensor power-of-2 scales, folded into constants):
  x*SX, w1*SW1, w3*SW3, w2*SW2 quantized to fp8 hi+lo.
  PSUM_g = SX*SW1 * g      -> silu input scale 1/(SX*SW1)
  PSUM_u = SX*SW3 * u      -> h' = silu(g) * PSUM_u = SX*SW3 * h
  h' quantized to fp8 hi+lo directly (|h'| < 240 by choice of SW3).
  PSUM_y = SX*SW3*SW2 * y  -> combine weight folded: cvec = c/(SX*SW3*SW2)
"""

import ml_dtypes
import numpy as np

import concourse.bass as bass  # noqa: F401  (engine plumbing)
import concourse.tile as tile
from concourse import bacc, mybir
from concourse.bass_utils import run_bass_kernel_spmd

T, H, F, E, TOPK = 4096, 1024, 512, 16, 4
NCORES = 8
F32 = mybir.dt.float32
BF16 = mybir.dt.bfloat16
FP8 = mybir.dt.float8e4
NPFP8 = ml_dtypes.float8_e4m3

SX, SW1, SW3, SW2 = 2.0, 64.0, 8.0, 64.0
SILU_SCALE = 1.0 / (SX * SW1)          # PSUM_g -> true g
CSCALE = 1.0 / (SX * SW3 * SW2)        # PSUM_y -> true y, folded into cvec

KP = H // 256    # stage-1 contraction k-pairs (DoubleRow: 256 per pair)
FPAIR = F // 256  # stage-2 contraction f-pairs

_nc_cache: dict = {}
LAST_CAPS = (1408, 1024)  # caps used by the most recent kernel() call


def _chunk_list(caps):
    """(slot, t0, tl) chunks of <=512 tokens covering all slots.

    The smallest chunk is moved last so the kernel's drain tail (final
    stage-2 + output DMA) is as short as possible.
    """
    out = []
    t0 = 0
    for s in range(len(caps)):
        rem = caps[s]
        while rem > 0:
            tl = min(512, rem)
            out.append((s, t0, tl))
            t0 += tl
            rem -= tl
    smallest = min(range(len(out)), key=lambda i: out[i][2])
    out.append(out.pop(smallest))
    return out


def _plan_slots(counts):
    """Choose per-core slot capacities + (expert, piece) -> slot assignment.

    Returns (caps, assign) where assign[core][slot] = (expert, lo, hi) token
    sub-range of that expert's token list (or None), minimizing the static
    per-core token capacity sum(caps).

    Plan A (always feasible): slot0 = 8 heaviest experts, slot1 = 8
    lightest; caps = (pad(max heavy), pad(max light)).
    Plan B: two 1024-token base slots + one small overflow slot holding the
    spill of experts that exceed 1024 (at most one spill piece per core).
    """
    E_ = len(counts)
    order = sorted(range(E_), key=lambda e: -counts[e])
    ncores = NCORES

    # Plan A
    cap_a0 = _pad128(max(counts[e] for e in order[:ncores]))
    cap_b = _pad128(max(counts[e] for e in order[ncores:]))
    capsA = (cap_a0, cap_b)
    def piece(e, lo, hi):
        return (e, lo, hi) if hi > lo else None

    assignA = []
    for c in range(ncores):
        e0, e1 = order[c], order[ncores + c]
        assignA.append([piece(e0, 0, counts[e0]), piece(e1, 0, counts[e1])])

    best_caps, best_assign = capsA, assignA

    # Plan B: base slots (base_a, cap_b) + small overflow slot cap_c; the
    # spill of experts exceeding base_a is cut into <=cap_c pieces, one
    # piece per core's overflow slot (<= 8 pieces total).
    for base_a in range(1024, cap_a0, 128):
        for cap_c in (128, 256, 384):
            if base_a + cap_b + cap_c >= sum(best_caps):
                continue
            spills = []
            for e in order[:ncores]:
                rem = counts[e] - base_a
                lo = base_a
                while rem > 0:
                    take = min(cap_c, rem)
                    spills.append((e, lo, lo + take))
                    lo += take
                    rem -= take
            if len(spills) > ncores:
                continue
            assignB = []
            for c in range(ncores):
                e0, e1 = order[c], order[ncores + c]
                row = [piece(e0, 0, min(counts[e0], base_a)),
                       piece(e1, 0, counts[e1]),
                       spills[c] if c < len(spills) else None]
                assignB.append(row)
            best_caps, best_assign = (base_a, cap_b, cap_c), assignB

    # validate: every expert fully covered by in-cap pieces; else Plan A
    covered = [0] * E_
    ok = True
    for row in best_assign:
        for s, piece in enumerate(row):
            if piece is None:
                continue
            e, lo, hi = piece
            ok = ok and 0 < hi - lo <= best_caps[s]
            covered[e] += hi - lo
    if not ok or covered != list(counts):
        best_caps, best_assign = capsA, assignA
    return best_caps, best_assign


def _build_phase_b(caps: tuple):
    """Expert FFN, fp8 DoubleRow with hi/lo error compensation.

    Inputs per core (NS = number of expert slots):
      w13q [NS, 2, H, 2F] per-slot, per-(hi,lo) hstack(w1[e].T*SW1, w3[e].T*SW3)
      w2q  [NS, 2, F, H]  per-slot, per-(hi,lo) w2[e].T*SW2
      xgq  [2, H, CT]     per-(hi,lo) gathered tokens (transposed), fp8
      cvec [128, CT/128]  combine weight * CSCALE per gathered token
    Output:
      yg   [CT, H]        combine-weighted expert outputs, bf16
    """
    DR = mybir.MatmulPerfMode.DoubleRow
    SILU = mybir.ActivationFunctionType.Silu
    COPY = mybir.ActivationFunctionType.Copy
    NS = len(caps)
    CT = sum(caps)
    assert CT % 128 == 0
    nc = bacc.Bacc("TRN2", target_bir_lowering=False, debug=False,
                   num_devices=NCORES)
    w13q = nc.dram_tensor("w13q", [NS, 2, H, 2 * F], FP8,
                          kind="ExternalInput").ap()
    w2q = nc.dram_tensor("w2q", [NS, 2, F, H], FP8, kind="ExternalInput").ap()
    xgq = nc.dram_tensor("xgq", [2, H, CT], FP8, kind="ExternalInput").ap()
    cvec = nc.dram_tensor("cvec", [128, CT // 128], F32,
                          kind="ExternalInput").ap()
    yg = nc.dram_tensor("yg", [CT, H], BF16, kind="ExternalOutput").ap()

    chunks = _chunk_list(caps)

    with tile.TileContext(nc) as tc:
        with (
            tc.tile_pool(name="w13_p", bufs=1) as w13_p,
            tc.tile_pool(name="w2_p", bufs=1) as w2_p,
            tc.tile_pool(name="xg_p", bufs=2) as xg_p,
            tc.tile_pool(name="sg_p", bufs=2) as sg_p,
            tc.tile_pool(name="hp_p", bufs=2) as hp_p,
            tc.tile_pool(name="hq_p", bufs=2) as hq_p,
            tc.tile_pool(name="y_p", bufs=2) as y_p,
            tc.tile_pool(name="c_p", bufs=1) as c_p,
            tc.tile_pool(name="ps", bufs=8, space="PSUM") as ps_pool,
        ):
            c_sb = c_p.tile([128, CT // 128], F32)

            # Weights. w13 hi split per k-pair so the first matmuls only wait
            # on a 256KB DMA; lo and w2 arrive while hi*hi matmuls run.
            whi13 = [[w13_p.tile([128, 2, 2 * F], FP8, name=f"whi13_{s}_{kp}")
                      for kp in range(KP)] for s in range(NS)]
            wlo13 = [[w13_p.tile([128, 2, 2 * F], FP8, name=f"wlo13_{s}_{kp}")
                      for kp in range(KP)] for s in range(NS)]
            whi2 = [w2_p.tile([128, FPAIR, 2, H], FP8, name=f"whi2_{s}")
                    for s in range(NS)]
            wlo2 = [w2_p.tile([128, FPAIR, 2, H], FP8, name=f"wlo2_{s}")
                    for s in range(NS)]

            def load_w13(s, hi):
                tiles, v = (whi13, 0) if hi else (wlo13, 1)
                eng = nc.sync if hi else nc.gpsimd
                for kp in range(KP):
                    eng.dma_start(
                        tiles[s][kp][:],
                        w13q[s, v, kp * 256:(kp + 1) * 256, :].rearrange(
                            "(two p) f -> p two f", p=128))

            def load_w2(s):
                nc.gpsimd.dma_start(
                    whi2[s][:],
                    w2q[s, 0].rearrange("(fp two p) h -> p fp two h", p=128, two=2))
                nc.gpsimd.dma_start(
                    wlo2[s][:],
                    w2q[s, 1].rearrange("(fp two p) h -> p fp two h", p=128, two=2))

            # PE clock warm-up: dummy matmuls on a memset tile while the
            # first weight/activation DMAs stream in (the p-state model
            # upclocks after ~3us of continuous PE activity).
            warm = c_p.tile([128, 2, 128], FP8, name="warm")
            nc.gpsimd.memset(warm[:], 0)
            ps_warm = ps_pool.tile([128, 128], F32, tag="ps", name="ps_warm")
            for _ in range(44):
                nc.tensor.matmul(ps_warm[:], lhsT=warm[:], rhs=warm[:],
                                 start=True, stop=True, perf_mode=DR)



            xgq_r = [xgq[v].rearrange("(kp two p) t -> p kp two t", p=128, two=2)
                     for v in range(2)]

            def stage2(s, t0, tl, hq_hi, hq_lo, last=False):
                y_sb = y_p.tile([128, 4, H], BF16, tag="y",
                                name=f"y_{t0}")
                for tt0 in range(0, tl, 128):
                    cidx = (t0 + tt0) // 128
                    for hh in range(2):
                        ps_y = ps_pool.tile([128, 512], F32, tag="ps",
                                            name=f"psy_{t0}_{tt0}_{hh}")
                        idx = 0
                        for ht, wt in ((hq_hi, whi2[s]), (hq_hi, wlo2[s]),
                                       (hq_lo, whi2[s])):
                            for fp in range(FPAIR):
                                nc.tensor.matmul(
                                    ps_y[:],
                                    lhsT=ht[:, fp, :, tt0:tt0 + 128],
                                    rhs=wt[:, fp, :, hh * 512:(hh + 1) * 512],
                                    start=(idx == 0),
                                    stop=(idx == 3 * FPAIR - 1),
                                    perf_mode=DR)
                                idx += 1
                        if last:
                            # drain tail: per-half tiles (no shared-tile WAW
                            # between the two scale engines), scaled on
                            # ACT/DVE in parallel, shipped from two queues.
                            final = tt0 + 128 >= tl and hh == 1
                            if final:
                                # chain-critical last piece: quarter scales
                                # on ACT+DVE in parallel (read-read on the
                                # same PSUM group is safe; separate output
                                # tiles avoid the tile-granular WAW), two
                                # parallel-queue DMAs
                                ya = y_p.tile([128, 256], BF16, tag="yta",
                                              name=f"yta_{t0}_{tt0}")
                                yb = y_p.tile([128, 256], BF16, tag="ytb",
                                              name=f"ytb_{t0}_{tt0}")
                                nc.scalar.activation(
                                    ya[:], ps_y[:, :256], COPY,
                                    scale=c_sb[:, cidx:cidx + 1])
                                nc.vector.tensor_scalar(
                                    yb[:], ps_y[:, 256:],
                                    c_sb[:, cidx:cidx + 1], None,
                                    op0=mybir.AluOpType.mult)
                                nc.gpsimd.dma_start(
                                    yg[t0 + tt0:t0 + tt0 + 128,
                                       hh * 512:hh * 512 + 256], ya[:])
                                nc.sync.dma_start(
                                    yg[t0 + tt0:t0 + tt0 + 128,
                                       hh * 512 + 256:(hh + 1) * 512], yb[:])
                                continue
                            yh = y_p.tile([128, 512], BF16, tag=f"yt{hh}",
                                          name=f"ytail_{t0}_{tt0}_{hh}")
                            if hh == 1:
                                nc.vector.tensor_scalar(
                                    yh[:], ps_y[:], c_sb[:, cidx:cidx + 1],
                                    None, op0=mybir.AluOpType.mult)
                            else:
                                nc.scalar.activation(
                                    yh[:], ps_y[:], COPY,
                                    scale=c_sb[:, cidx:cidx + 1])
                            eng = nc.gpsimd if hh == 0 else nc.sync
                            eng.dma_start(
                                yg[t0 + tt0:t0 + tt0 + 128,
                                   hh * 512:(hh + 1) * 512], yh[:])
                            continue
                        ydst = y_sb[:, tt0 // 128, hh * 512:(hh + 1) * 512]
                        nc.scalar.activation(
                            ydst, ps_y[:], COPY,
                            scale=c_sb[:, cidx:cidx + 1])
                    # ship each 128-token tile as soon as it is scaled, so
                    # output traffic overlaps compute instead of piling up
                    # at the drain; alternate queues to halve serialization
                    if not last:
                        eng = nc.sync if ((t0 + tt0) // 128) % 2 else nc.gpsimd
                        eng.dma_start(
                            yg[t0 + tt0:t0 + tt0 + 128], y_sb[:, tt0 // 128])

            pending = None
            for ci, (s, t0, tl) in enumerate(chunks):
                xhi = xg_p.tile([128, KP, 2, 512], FP8, tag="xhi",
                                name=f"xhi_{ci}")
                xlo = xg_p.tile([128, KP, 2, 512], FP8, tag="xlo",
                                name=f"xlo_{ci}")
                if ci == 0:
                    # per-k-pair pieces on two parallel DMA channels: weights
                    # on the SP queue, activations on the scalar queue, so
                    # the k-th hi*hi wave starts after two small transfers.
                    # The very first 128 lhsT columns go in a tiny DMA so
                    # matmul 0 fires as early as possible.
                    nc.sync.dma_start(
                        whi13[s][0][:, :, :128],
                        w13q[s, 0, :256, :128].rearrange(
                            "(two p) f -> p two f", p=128))
                    nc.sync.dma_start(
                        whi13[s][0][:, :, 128:],
                        w13q[s, 0, :256, 128:].rearrange(
                            "(two p) f -> p two f", p=128))
                    for kp in range(1, KP):
                        nc.sync.dma_start(
                            whi13[s][kp][:],
                            w13q[s, 0, kp * 256:(kp + 1) * 256, :].rearrange(
                                "(two p) f -> p two f", p=128))
                    for kp in range(KP):
                        nc.scalar.dma_start(xhi[:, kp, :, :tl],
                                            xgq_r[0][:, kp, :, t0:t0 + tl])
                else:
                    nc.sync.dma_start(xhi[:, :, :, :tl],
                                      xgq_r[0][:, :, :, t0:t0 + tl])
                if ci == 0:
                    # halves on the otherwise-idle scalar queue: the hi*lo
                    # wave then only waits on these, not on wlo13 (waits are
                    # coalesced per queue)
                    nc.scalar.dma_start(xlo[:, :2, :, :tl],
                                        xgq_r[1][:, :2, :, t0:t0 + tl])
                    nc.scalar.dma_start(xlo[:, 2:, :, :tl],
                                        xgq_r[1][:, 2:, :, t0:t0 + tl])
                else:
                    nc.scalar.dma_start(xlo[:, :, :, :tl],
                                        xgq_r[1][:, :, :, t0:t0 + tl])
                if ci == 0:
                    # chunk-0 needs only its own slot's w13 (hi first; the
                    # hi*hi products run while xlo/wlo13 stream in)
                    load_w13(s, hi=False)
                    nc.gpsimd.dma_start(c_sb[:], cvec[:])
                    w13_loaded = {s}
                    w2_loaded = set()
                else:
                    # demand-staged prefetch: next chunk's w13, current and
                    # next slot's w2, each about one chunk ahead of first use
                    nxt = chunks[ci + 1][0] if ci + 1 < len(chunks) else None
                    if s not in w13_loaded:
                        load_w13(s, hi=True)
                        load_w13(s, hi=False)
                        w13_loaded.add(s)
                    if s not in w2_loaded:
                        load_w2(s)
                        w2_loaded.add(s)
                    if nxt is not None and nxt not in w13_loaded:
                        load_w13(nxt, hi=True)
                        load_w13(nxt, hi=False)
                        w13_loaded.add(nxt)
                    if nxt is not None and nxt not in w2_loaded:
                        load_w2(nxt)
                        w2_loaded.add(nxt)

                hq_hi = hq_p.tile([128, FPAIR, 2, 512], FP8, tag="hqhi",
                                  name=f"hqhi_{ci}")
                hq_lo = hq_p.tile([128, FPAIR, 2, 512], FP8, tag="hqlo",
                                  name=f"hqlo_{ci}")
                ps_g = [ps_pool.tile([128, 512], F32, tag="ps",
                                     name=f"psg_{ci}_{fi}") for fi in range(4)]
                ps_u = [ps_pool.tile([128, 512], F32, tag="ps",
                                     name=f"psu_{ci}_{fi}") for fi in range(4)]

                def mm_s1(prod, fi, path, kp, first, last):
                    wt, xt = ((whi13[s], xhi), (whi13[s], xlo),
                              (wlo13[s], xhi))[prod]
                    ps = (ps_g, ps_u)[path][fi]
                    col0 = path * F + fi * 128
                    nc.tensor.matmul(
                        ps[:, :tl], lhsT=wt[kp][:, :, col0:col0 + 128],
                        rhs=xt[:, kp, :, :tl], start=first, stop=last,
                        perf_mode=DR)

                if ci == 0:
                    # hi*hi first across all groups, kp-major (each kp wave
                    # needs only one small weight+activation DMA pair); then
                    # finish each group in turn so PSUM banks free
                    # progressively
                    for kp in range(KP):
                        for fi in range(4):
                            for path in range(2):
                                mm_s1(0, fi, path, kp, kp == 0, False)
                    for fi in range(4):
                        for path in range(2):
                            for prod in (1, 2):
                                for kp in range(KP):
                                    mm_s1(prod, fi, path, kp, False,
                                          prod == 2 and kp == KP - 1)
                else:
                    for fi in range(4):
                        for path in range(2):
                            idx = 0
                            for prod in range(3):
                                for kp in range(KP):
                                    mm_s1(prod, fi, path, kp, idx == 0,
                                          idx == 3 * KP - 1)
                                    idx += 1

                for fi in range(4):
                    fp, two = fi // 2, fi % 2
                    sg = sg_p.tile([128, 512], F32, tag="sg",
                                   name=f"sg_{ci}_{fi}")
                    nc.scalar.activation(sg[:, :tl], ps_g[fi][:, :tl], SILU,
                                         scale=SILU_SCALE)
                    hp = hp_p.tile([128, 512], F32, tag="hp",
                                   name=f"hp_{ci}_{fi}")
                    nc.vector.tensor_mul(hp[:, :tl], sg[:, :tl],
                                         ps_u[fi][:, :tl])
                    nc.vector.tensor_copy(hq_hi[:, fp, two, :tl], hp[:, :tl])
                    nc.vector.tensor_sub(hq_lo[:, fp, two, :tl], hp[:, :tl],
                                         hq_hi[:, fp, two, :tl])

                if pending is not None:
                    stage2(*pending)
                pending = (s, t0, tl, hq_hi, hq_lo)
            stage2(*pending, last=True)

    nc.compile()
    return nc


def _phase_b_nc(caps):
    key = ("b", caps)
    if key not in _nc_cache:
        _nc_cache[key] = _build_phase_b(caps)
    return _nc_cache[key]


def _pad128(n: int) -> int:
    return max(128, (n + 127) // 128 * 128)


def _hilo(a: np.ndarray, scale: float):
    """fp8-e4m3 hi/lo decomposition of a*scale (hi + lo ~= a*scale)."""
    s = (a * scale).astype(np.float32)
    hi = s.astype(NPFP8)
    lo = (s - hi.astype(np.float32)).astype(NPFP8)
    return hi, lo


def kernel(hidden_states, gate_w, bias, w1, w3, w2):
    x = np.ascontiguousarray(np.asarray(hidden_states, dtype=np.float32))
    gate_w = np.asarray(gate_w, dtype=np.float32)
    bias = np.asarray(bias, dtype=np.float32)
    w1 = np.asarray(w1, dtype=np.float32)
    w3 = np.asarray(w3, dtype=np.float32)
    w2 = np.asarray(w2, dtype=np.float32)

    # ----

    # Routing on host, f32 (matches reference math; top-k ties -> lower idx).
    logits = x @ gate_w.T                               # [T, E]
    scores = 1.0 / (1.0 + np.exp(-logits))
    topi = np.argsort(-(scores + bias[None, :]), axis=1,
                      kind="stable")[:, :TOPK]          # [T, K]
    topw = np.take_along_axis(scores, topi, axis=1)
    topw = topw / topw.sum(axis=1, keepdims=True)
    combine = np.zeros((T, E), dtype=np.float32)
    np.put_along_axis(combine, topi, topw, axis=1)      # [T, E]

    # ---- Host dispatch: slot plan (expert pieces -> core slots) ----
    idx_per_e = [np.nonzero(combine[:, e] > 0.0)[0] for e in range(E)]
    counts = [len(ix) for ix in idx_per_e]
    caps, assign = _plan_slots(counts)
    global LAST_CAPS
    LAST_CAPS = caps
    NS = len(caps)
    CT = sum(caps)
    offs = [sum(caps[:s]) for s in range(NS)]

    # ---- Host quantization: fp8 hi/lo of activations and weights ----
    xT = np.ascontiguousarray(x.T)                      # [H, T]
    xhi, xlo = _hilo(xT, SX)

    w13q_per_e = {}
    w2q_per_e = {}
    for e in set(p[0] for row in assign for p in row if p is not None):
        w13 = np.concatenate([w1[e].T * SW1, w3[e].T * SW3], axis=1)
        w13q_per_e[e] = _hilo(w13, 1.0)
        w2q_per_e[e] = _hilo(w2[e].T, SW2)

    in_maps = []
    for c in range(NCORES):
        idx_pad = np.zeros(CT, dtype=np.int64)
        cv = np.zeros(CT, dtype=np.float32)
        w13q = np.zeros((NS, 2, H, 2 * F), dtype=NPFP8)
        w2q = np.zeros((NS, 2, F, H), dtype=NPFP8)
        for s, piece in enumerate(assign[c]):
            if piece is None:
                continue
            e, lo_i, hi_i = piece
            ix = idx_per_e[e][lo_i:hi_i]
            idx_pad[offs[s]:offs[s] + len(ix)] = ix
            cv[offs[s]:offs[s] + len(ix)] = combine[ix, e] * CSCALE
            w13q[s, 0], w13q[s, 1] = w13q_per_e[e]
            w2q[s, 0], w2q[s, 1] = w2q_per_e[e]
        xgq = np.stack([np.ascontiguousarray(xhi[:, idx_pad]),
                        np.ascontiguousarray(xlo[:, idx_pad])])  # [2, H, CT]
        cv_tiled = np.ascontiguousarray(cv.reshape(CT // 128, 128).T)
        in_maps.append({"w13q": w13q, "w2q": w2q, "xgq": xgq,
                        "cvec": cv_tiled})

    # ---- Phase B: expert FFN on device (expert-parallel) ----
    ncB = _phase_b_nc(caps)
    resB = run_bass_kernel_spmd(ncB, in_maps, core_ids=list(range(NCORES)))

    # ---- Host combine: scatter-add in expert order ----
    out = np.zeros((T, H), dtype=np.float32)
    pieces_by_e = [[] for _ in range(E)]
    for c in range(NCORES):
        for s, piece in enumerate(assign[c]):
            if piece is None:
                continue
            e, lo_i, hi_i = piece
            pieces_by_e[e].append((c, offs[s], lo_i, hi_i))
    for e in range(E):
        for c, off, lo_i, hi_i in pieces_by_e[e]:
            ix = idx_per_e[e][lo_i:hi_i]
            if len(ix):
                out[ix] += resB.results[c]["yg"][off:off + len(ix)
                                                 ].astype(np.float32)
    return out


# revision 44
# speedup vs baseline: 1.0032x; 1.0032x over previous
"""MiniMax-M2 MoE kernel for 8 Trainium2 NeuronCores.

Strategy (expert-parallel, sparse/routed):
  Host: router gate matmul + sigmoid + top-4 selection + combine-weight
    renormalization in f32 numpy (pure data movement / tiny matmul), then
    gather tokens per expert, pad each expert slot to a static capacity.
  Host: quantize activations and weights to fp8-e4m3 hi/lo pairs
    (value = hi + lo exactly to ~2^-14 relative) so the device can run all
    matmuls in DoubleRow fp8 perf mode while keeping ~bf16 accuracy via
    3-product error compensation (hi*hi + hi*lo + lo*hi; lo*lo dropped).
  Device (expert-parallel): per core, 2-3 expert slots' SwiGLU FFN over
    the gathered tokens (slot plan chosen to minimize the static per-core
    token capacity; heavy experts may spill a small slice into a third
    slot on another core); combine weight applied on device; bf16 outputs.
  Host: scatter-add per-expert outputs into the [T, H] result in expert
    order (matches the reference scan accumulation order).

Scale bookkeeping (per-tensor power-of-2 scales, folded into constants):
  x*SX, w1*SW1, w3*SW3, w2*SW2 quantized to fp8 hi+lo.
  PSUM_g = SX*SW1 * g      -> silu input scale 1/(SX*SW1)
  PSUM_u = SX*SW3 * u      -> h' = silu(g) * PSUM_u = SX*SW3 * h
  h' quantized to fp8 hi+lo directly (|h'| < 240 by choice of SW3).
  PSUM_y = SX*SW3*SW2 * y  -> combine weight folded: cvec = c/(SX*SW3*SW2)
"""

import ml_dtypes
import numpy as np

import concourse.bass as bass  # noqa: F401  (engine plumbing)
import concourse.tile as tile
from concourse import bacc, mybir
from concourse.bass_utils import run_bass_kernel_spmd

T, H, F, E, TOPK = 4096, 1024, 512, 16, 4
NCORES = 8
F32 = mybir.dt.float32
BF16 = mybir.dt.bfloat16
FP8 = mybir.dt.float8e4
NPFP8 = ml_dtypes.float8_e4m3

SX, SW1, SW3, SW2 = 2.0, 64.0, 8.0, 64.0
SILU_SCALE = 1.0 / (SX * SW1)          # PSUM_g -> true g
CSCALE = 1.0 / (SX * SW3 * SW2)        # PSUM_y -> true y, folded into cvec

KP = H // 256    # stage-1 contraction k-pairs (DoubleRow: 256 per pair)
FPAIR = F // 256  # stage-2 contraction f-pairs

_nc_cache: dict = {}
LAST_CAPS = (1408, 1024)  # caps used by the most recent kernel() call


def _chunk_list(caps):
    """(slot, t0, tl) chunks of <=512 tokens covering all slots.

    The smallest chunk is moved last so the kernel's drain tail (final
    stage-2 + output DMA) is as short as possible.
    """
    out = []
    t0 = 0
    for s in range(len(caps)):
        rem = caps[s]
        while rem > 0:
            tl = min(512, rem)
            out.append((s, t0, tl))
            t0 += tl
            rem -= tl
    smallest = min(range(len(out)), key=lambda i: out[i][2])
    out.append(out.pop(smallest))
    return out


def _plan_slots(counts):
    """Choose per-core slot capacities + (expert, piece) -> slot assignment.

    Returns (caps, assign) where assign[core][slot] = (expert, lo, hi) token
    sub-range of that expert's token list (or None), minimizing the static
    per-core token capacity sum(caps).

    Plan A (always feasible): slot0 = 8 heaviest experts, slot1 = 8
    lightest; caps = (pad(max heavy), pad(max light)).
    Plan B: two 1024-token base slots + one small overflow slot holding the
    spill of experts that exceed 1024 (at most one spill piece per core).
    """
    E_ = len(counts)
    order = sorted(range(E_), key=lambda e: -counts[e])
    ncores = NCORES

    # Plan A
    cap_a0 = _pad128(max(counts[e] for e in order[:ncores]))
    cap_b = _pad128(max(counts[e] for e in order[ncores:]))
    capsA = (cap_a0, cap_b)
    def piece(e, lo, hi):
        return (e, lo, hi) if hi > lo else None

    assignA = []
    for c in range(ncores):
        e0, e1 = order[c], order[ncores + c]
        assignA.append([piece(e0, 0, counts[e0]), piece(e1, 0, counts[e1])])

    best_caps, best_assign = capsA, assignA

    # Plan B: base slots (base_a, cap_b) + small overflow slot cap_c; the
    # spill of experts exceeding base_a is cut into <=cap_c pieces, one
    # piece per core's overflow slot (<= 8 pieces total).
    for base_a in range(1024, cap_a0, 128):
        for cap_c in (128, 256, 384):
            if base_a + cap_b + cap_c >= sum(best_caps):
                continue
            spills = []
            for e in order[:ncores]:
                rem = counts[e] - base_a
                lo = base_a
                while rem > 0:
                    take = min(cap_c, rem)
                    spills.append((e, lo, lo + take))
                    lo += take
                    rem -= take
            if len(spills) > ncores:
                continue
            assignB = []
            for c in range(ncores):
                e0, e1 = order[c], order[ncores + c]
                row = [piece(e0, 0, min(counts[e0], base_a)),
                       piece(e1, 0, counts[e1]),
                       spills[c] if c < len(spills) else None]
                assignB.append(row)
            best_caps, best_assign = (base_a, cap_b, cap_c), assignB

    # validate: every expert fully covered by in-cap pieces; else Plan A
    covered = [0] * E_
    ok = True
    for row in best_assign:
        for s, piece in enumerate(row):
            if piece is None:
                continue
            e, lo, hi = piece
            ok = ok and 0 < hi - lo <= best_caps[s]
            covered[e] += hi - lo
    if not ok or covered != list(counts):
        best_caps, best_assign = capsA, assignA
    return best_caps, best_assign


def _build_phase_b(caps: tuple):
    """Expert FFN, fp8 DoubleRow with hi/lo error compensation.

    Inputs per core (NS = number of expert slots):
      w13q [NS, 2, H, 2F] per-slot, per-(hi,lo) hstack(w1[e].T*SW1, w3[e].T*SW3)
      w2q  [NS, 2, F, H]  per-slot, per-(hi,lo) w2[e].T*SW2
      xgq  [2, H, CT]     per-(hi,lo) gathered tokens (transposed), fp8
      cvec [128, CT/128]  combine weight * CSCALE per gathered token
    Output:
      yg   [CT, H]        combine-weighted expert outputs, bf16
    """
    DR = mybir.MatmulPerfMode.DoubleRow
    SILU = mybir.ActivationFunctionType.Silu
    COPY = mybir.ActivationFunctionType.Copy
    NS = len(caps)
    CT = sum(caps)
    assert CT % 128 == 0
    nc = bacc.Bacc("TRN2", target_bir_lowering=False, debug=False,
                   num_devices=NCORES)
    w13q = nc.dram_tensor("w13q", [NS, 2, H, 2 * F], FP8,
                          kind="ExternalInput").ap()
    w2q = nc.dram_tensor("w2q", [NS, 2, F, H], FP8, kind="ExternalInput").ap()
    xgq = nc.dram_tensor("xgq", [2, H, CT], FP8, kind="ExternalInput").ap()
    cvec = nc.dram_tensor("cvec", [128, CT // 128], F32,
                          kind="ExternalInput").ap()
    yg = nc.dram_tensor("yg", [CT, H], BF16, kind="ExternalOutput").ap()

    chunks = _chunk_list(caps)

    with tile.TileContext(nc) as tc:
        with (
            tc.tile_pool(name="w13_p", bufs=1) as w13_p,
            tc.tile_pool(name="w2_p", bufs=1) as w2_p,
            tc.tile_pool(name="xg_p", bufs=2) as xg_p,
            tc.tile_pool(name="sg_p", bufs=2) as sg_p,
            tc.tile_pool(name="hp_p", bufs=2) as hp_p,
            tc.tile_pool(name="hq_p", bufs=2) as hq_p,
            tc.tile_pool(name="y_p", bufs=2) as y_p,
            tc.tile_pool(name="c_p", bufs=1) as c_p,
            tc.tile_pool(name="ps", bufs=8, space="PSUM") as ps_pool,
        ):
            c_sb = c_p.tile([128, CT // 128], F32)

            # Weights. w13 hi split per k-pair so the first matmuls only wait
            # on a 256KB DMA; lo and w2 arrive while hi*hi matmuls run.
            whi13 = [[w13_p.tile([128, 2, 2 * F], FP8, name=f"whi13_{s}_{kp}")
                      for kp in range(KP)] for s in range(NS)]
            wlo13 = [[w13_p.tile([128, 2, 2 * F], FP8, name=f"wlo13_{s}_{kp}")
                      for kp in range(KP)] for s in range(NS)]
            whi2 = [w2_p.tile([128, FPAIR, 2, H], FP8, name=f"whi2_{s}")
                    for s in range(NS)]
            wlo2 = [w2_p.tile([128, FPAIR, 2, H], FP8, name=f"wlo2_{s}")
                    for s in range(NS)]

            def load_w13(s, hi):
                tiles, v = (whi13, 0) if hi else (wlo13, 1)
                eng = nc.sync if hi else nc.gpsimd
                for kp in range(KP):
                    eng.dma_start(
                        tiles[s][kp][:],
                        w13q[s, v, kp * 256:(kp + 1) * 256, :].rearrange(
                            "(two p) f -> p two f", p=128))

            def load_w2(s):
                nc.gpsimd.dma_start(
                    whi2[s][:],
                    w2q[s, 0].rearrange("(fp two p) h -> p fp two h", p=128, two=2))
                nc.gpsimd.dma_start(
                    wlo2[s][:],
                    w2q[s, 1].rearrange("(fp two p) h -> p fp two h", p=128, two=2))

            # PE clock warm-up: dummy matmuls on a memset tile while the
            # first weight/activation DMAs stream in (the p-state model
            # upclocks after ~3us of continuous PE activity).
            warm = c_p.tile([128, 2, 128], FP8, name="warm")
            nc.gpsimd.memset(warm[:], 0)
            ps_warm = ps_pool.tile([128, 128], F32, tag="ps", name="ps_warm")
            for _ in range(44):
                nc.tensor.matmul(ps_warm[:], lhsT=warm[:], rhs=warm[:],
                                 start=True, stop=True, perf_mode=DR)



            xgq_r = [xgq[v].rearrange("(kp two p) t -> p kp two t", p=128, two=2)
                     for v in range(2)]

            def stage2(s, t0, tl, hq_hi, hq_lo, last=False):
                y_sb = y_p.tile([128, 4, H], BF16, tag="y",
                                name=f"y_{t0}")
                for tt0 in range(0, tl, 128):
                    cidx = (t0 + tt0) // 128
                    for hh in range(2):
                        ps_y = ps_pool.tile([128, 512], F32, tag="ps",
                                            name=f"psy_{t0}_{tt0}_{hh}")
                        idx = 0
                        for ht, wt in ((hq_hi, whi2[s]), (hq_hi, wlo2[s]),
                                       (hq_lo, whi2[s])):
                            for fp in range(FPAIR):
                                nc.tensor.matmul(
                                    ps_y[:],
                                    lhsT=ht[:, fp, :, tt0:tt0 + 128],
                                    rhs=wt[:, fp, :, hh * 512:(hh + 1) * 512],
                                    start=(idx == 0),
                                    stop=(idx == 3 * FPAIR - 1),
                                    perf_mode=DR)
                                idx += 1
                        if last:
                            # drain tail: per-half tiles (no shared-tile WAW
                            # between the two scale engines), scaled on
                            # ACT/DVE in parallel, shipped from two queues.
                            # The chain-critical hh=1 half goes on the
                            # lower-latency SP queue.
                            yh = y_p.tile([128, 512], BF16, tag=f"yt{hh}",
                                          name=f"ytail_{t0}_{tt0}_{hh}")
                            final = tt0 + 128 >= tl and hh == 1
                            if hh == 1 and not final:
                                nc.vector.tensor_scalar(
                                    yh[:], ps_y[:], c_sb[:, cidx:cidx + 1],
                                    None, op0=mybir.AluOpType.mult)
                            else:
                                # ACT is slightly faster per element; use it
                                # for the chain-critical final piece too
                                nc.scalar.activation(
                                    yh[:], ps_y[:], COPY,
                                    scale=c_sb[:, cidx:cidx + 1])
                            eng = nc.gpsimd if hh == 0 else nc.sync
                            eng.dma_start(
                                yg[t0 + tt0:t0 + tt0 + 128,
                                   hh * 512:(hh + 1) * 512], yh[:])
                            continue
                        ydst = y_sb[:, tt0 // 128, hh * 512:(hh + 1) * 512]
                        nc.scalar.activation(
                            ydst, ps_y[:], COPY,
                            scale=c_sb[:, cidx:cidx + 1])
                    # ship each 128-token tile as soon as it is scaled, so
                    # output traffic overlaps compute instead of piling up
                    # at the drain; alternate queues to halve serialization
                    if not last:
                        eng = nc.sync if ((t0 + tt0) // 128) % 2 else nc.gpsimd
                        eng.dma_start(
                            yg[t0 + tt0:t0 + tt0 + 128], y_sb[:, tt0 // 128])

            pending = None
            for ci, (s, t0, tl) in enumerate(chunks):
                xhi = xg_p.tile([128, KP, 2, 512], FP8, tag="xhi",
                                name=f"xhi_{ci}")
                xlo = xg_p.tile([128, KP, 2, 512], FP8, tag="xlo",
                                name=f"xlo_{ci}")
                if ci == 0:
                    # per-k-pair pieces on two parallel DMA channels: weights
                    # on the SP queue, activations on the scalar queue, so
                    # the k-th hi*hi wave starts after two small transfers.
                    # The very first 128 lhsT columns go in a tiny DMA so
                    # matmul 0 fires as early as possible.
                    nc.sync.dma_start(
                        whi13[s][0][:, :, :128],
                        w13q[s, 0, :256, :128].rearrange(
                            "(two p) f -> p two f", p=128))
                    nc.sync.dma_start(
                        whi13[s][0][:, :, 128:],
                        w13q[s, 0, :256, 128:].rearrange(
                            "(two p) f -> p two f", p=128))
                    for kp in range(1, KP):
                        nc.sync.dma_start(
                            whi13[s][kp][:],
                            w13q[s, 0, kp * 256:(kp + 1) * 256, :].rearrange(
                                "(two p) f -> p two f", p=128))
                    for kp in range(KP):
                        nc.scalar.dma_start(xhi[:, kp, :, :tl],
                                            xgq_r[0][:, kp, :, t0:t0 + tl])
                else:
                    nc.sync.dma_start(xhi[:, :, :, :tl],
                                      xgq_r[0][:, :, :, t0:t0 + tl])
                if ci == 0:
                    # halves on the otherwise-idle scalar queue: the hi*lo
                    # wave then only waits on these, not on wlo13 (waits are
                    # coalesced per queue)
                    nc.scalar.dma_start(xlo[:, :2, :, :tl],
                                        xgq_r[1][:, :2, :, t0:t0 + tl])
                    nc.scalar.dma_start(xlo[:, 2:, :, :tl],
                                        xgq_r[1][:, 2:, :, t0:t0 + tl])
                else:
                    nc.scalar.dma_start(xlo[:, :, :, :tl],
                                        xgq_r[1][:, :, :, t0:t0 + tl])
                if ci == 0:
                    # chunk-0 needs only its own slot's w13 (hi first; the
                    # hi*hi products run while xlo/wlo13 stream in)
                    load_w13(s, hi=False)
                    nc.gpsimd.dma_start(c_sb[:], cvec[:])
                    w13_loaded = {s}
                    w2_loaded = set()
                else:
                    # demand-staged prefetch: next chunk's w13, current and
                    # next slot's w2, each about one chunk ahead of first use
                    nxt = chunks[ci + 1][0] if ci + 1 < len(chunks) else None
                    if s not in w13_loaded:
                        load_w13(s, hi=True)
                        load_w13(s, hi=False)
                        w13_loaded.add(s)
                    if s not in w2_loaded:
                        load_w2(s)
                        w2_loaded.add(s)
                    if nxt is not None and nxt not in w13_loaded:
                        load_w13(nxt, hi=True)
                        load_w13(nxt, hi=False)
                        w13_loaded.add(nxt)
                    if nxt is not None and nxt not in w2_loaded:
                        load_w2(nxt)
                        w2_loaded.add(nxt)

                hq_hi = hq_p.tile([128, FPAIR, 2, 512], FP8, tag="hqhi",
                                  name=f"hqhi_{ci}")
                hq_lo = hq_p.tile([128, FPAIR, 2, 512], FP8, tag="hqlo",
                                  name=f"hqlo_{ci}")
                ps_g = [ps_pool.tile([128, 512], F32, tag="ps",
                                     name=f"psg_{ci}_{fi}") for fi in range(4)]
                ps_u = [ps_pool.tile([128, 512], F32, tag="ps",
                                     name=f"psu_{ci}_{fi}") for fi in range(4)]

                def mm_s1(prod, fi, path, kp, first, last):
                    wt, xt = ((whi13[s], xhi), (whi13[s], xlo),
                              (wlo13[s], xhi))[prod]
                    ps = (ps_g, ps_u)[path][fi]
                    col0 = path * F + fi * 128
                    nc.tensor.matmul(
                        ps[:, :tl], lhsT=wt[kp][:, :, col0:col0 + 128],
                        rhs=xt[:, kp, :, :tl], start=first, stop=last,
                        perf_mode=DR)

                if ci == 0:
                    # hi*hi first across all groups, kp-major (each kp wave
                    # needs only one small weight+activation DMA pair); then
                    # finish each group in turn so PSUM banks free
                    # progressively
                    for kp in range(KP):
                        for fi in range(4):
                            for path in range(2):
                                mm_s1(0, fi, path, kp, kp == 0, False)
                    for fi in range(4):
                        for path in range(2):
                            for prod in (1, 2):
                                for kp in range(KP):
                                    mm_s1(prod, fi, path, kp, False,
                                          prod == 2 and kp == KP - 1)
                else:
                    for fi in range(4):
                        for path in range(2):
                            idx = 0
                            for prod in range(3):
                                for kp in range(KP):
                                    mm_s1(prod, fi, path, kp, idx == 0,
                                          idx == 3 * KP - 1)
                                    idx += 1

                for fi in range(4):
                    fp, two = fi // 2, fi % 2
                    sg = sg_p.tile([128, 512], F32, tag="sg",
                                   name=f"sg_{ci}_{fi}")
                    nc.scalar.activation(sg[:, :tl], ps_g[fi][:, :tl], SILU,
                                         scale=SILU_SCALE)
                    hp = hp_p.tile([128, 512], F32, tag="hp",
                                   name=f"hp_{ci}_{fi}")
                    nc.vector.tensor_mul(hp[:, :tl], sg[:, :tl],
                                         ps_u[fi][:, :tl])
                    nc.vector.tensor_copy(hq_hi[:, fp, two, :tl], hp[:, :tl])
                    nc.vector.tensor_sub(hq_lo[:, fp, two, :tl], hp[:, :tl],
                                         hq_hi[:, fp, two, :tl])

                if pending is not None:
                    stage2(*pending)
                pending = (s, t0, tl, hq_hi, hq_lo)
            stage2(*pending, last=True)

    nc.compile()
    return nc


def _phase_b_nc(caps):
    key = ("b", caps)
    if key not in _nc_cache:
        _nc_cache[key] = _build_phase_b(caps)
    return _nc_cache[key]


def _pad128(n: int) -> int:
    return max(128, (n + 127) // 128 * 128)


def _hilo(a: np.ndarray, scale: float):
    """fp8-e4m3 hi/lo decomposition of a*scale (hi + lo ~= a*scale)."""
    s = (a * scale).astype(np.float32)
    hi = s.astype(NPFP8)
    lo = (s - hi.astype(np.float32)).astype(NPFP8)
    return hi, lo


def kernel(hidden_states, gate_w, bias, w1, w3, w2):
    x = np.ascontiguousarray(np.asarray(hidden_states, dtype=np.float32))
    gate_w = np.asarray(gate_w, dtype=np.float32)
    bias = np.asarray(bias, dtype=np.float32)
    w1 = np.asarray(w1, dtype=np.float32)
    w3 = np.asarray(w3, dtype=np.float32)
    w2 = np.asarray(w2, dtype=np.float32)

    # ----

    # Routing on host, f32 (matches reference math; top-k ties -> lower idx).
    logits = x @ gate_w.T                               # [T, E]
    scores = 1.0 / (1.0 + np.exp(-logits))
    topi = np.argsort(-(scores + bias[None, :]), axis=1,
                      kind="stable")[:, :TOPK]          # [T, K]
    topw = np.take_along_axis(scores, topi, axis=1)
    topw = topw / topw.sum(axis=1, keepdims=True)
    combine = np.zeros((T, E), dtype=np.float32)
    np.put_along_axis(combine, topi, topw, axis=1)      # [T, E]

    # ---- Host dispatch: slot plan (expert pieces -> core slots) ----
    idx_per_e = [np.nonzero(combine[:, e] > 0.0)[0] for e in range(E)]
    counts = [len(ix) for ix in idx_per_e]
    caps, assign = _plan_slots(counts)
    global LAST_CAPS
    LAST_CAPS = caps
    NS = len(caps)
    CT = sum(caps)
    offs = [sum(caps[:s]) for s in range(NS)]

    # ---- Host quantization: fp8 hi/lo of activations and weights ----
    xT = np.ascontiguousarray(x.T)                      # [H, T]
    xhi, xlo = _hilo(xT, SX)

    w13q_per_e = {}
    w2q_per_e = {}
    for e in set(p[0] for row in assign for p in row if p is not None):
        w13 = np.concatenate([w1[e].T * SW1, w3[e].T * SW3], axis=1)
        w13q_per_e[e] = _hilo(w13, 1.0)
        w2q_per_e[e] = _hilo(w2[e].T, SW2)

    in_maps = []
    for c in range(NCORES):
        idx_pad = np.zeros(CT, dtype=np.int64)
        cv = np.zeros(CT, dtype=np.float32)
        w13q = np.zeros((NS, 2, H, 2 * F), dtype=NPFP8)
        w2q = np.zeros((NS, 2, F, H), dtype=NPFP8)
        for s, piece in enumerate(assign[c]):
            if piece is None:
                continue
            e, lo_i, hi_i = piece
            ix = idx_per_e[e][lo_i:hi_i]
            idx_pad[offs[s]:offs[s] + len(ix)] = ix
            cv[offs[s]:offs[s] + len(ix)] = combine[ix, e] * CSCALE
            w13q[s, 0], w13q[s, 1] = w13q_per_e[e]
            w2q[s, 0], w2q[s, 1] = w2q_per_e[e]
        xgq = np.stack([np.ascontiguousarray(xhi[:, idx_pad]),
                        np.ascontiguousarray(xlo[:, idx_pad])])  # [2, H, CT]
        cv_tiled = np.ascontiguousarray(cv.reshape(CT // 128, 128).T)
        in_maps.append({"w13q": w13q, "w2q": w2q, "xgq": xgq,
                        "cvec": cv_tiled})

    # ---- Phase B: expert FFN on device (expert-parallel) ----
    ncB = _phase_b_nc(caps)
    resB = run_bass_kernel_spmd(ncB, in_maps, core_ids=list(range(NCORES)))

    # ---- Host combine: scatter-add in expert order ----
    out = np.zeros((T, H), dtype=np.float32)
    pieces_by_e = [[] for _ in range(E)]
    for c in range(NCORES):
        for s, piece in enumerate(assign[c]):
            if piece is None:
                continue
            e, lo_i, hi_i = piece
            pieces_by_e[e].append((c, offs[s], lo_i, hi_i))
    for e in range(E):
        for c, off, lo_i, hi_i in pieces_by_e[e]:
            ix = idx_per_e[e][lo_i:hi_i]
            if len(ix):
                out[ix] += resB.results[c]["yg"][off:off + len(ix)
                                                 ].astype(np.float32)
    return out


# revision 45
# speedup vs baseline: 1.0058x; 1.0026x over previous
"""MiniMax-M2 MoE kernel for 8 Trainium2 NeuronCores.

Strategy (expert-parallel, sparse/routed):
  Host: router gate matmul + sigmoid + top-4 selection + combine-weight
    renormalization in f32 numpy (pure data movement / tiny matmul), then
    gather tokens per expert, pad each expert slot to a static capacity.
  Host: quantize activations and weights to fp8-e4m3 hi/lo pairs
    (value = hi + lo exactly to ~2^-14 relative) so the device can run all
    matmuls in DoubleRow fp8 perf mode while keeping ~bf16 accuracy via
    3-product error compensation (hi*hi + hi*lo + lo*hi; lo*lo dropped).
  Device (expert-parallel): per core, 2-3 expert slots' SwiGLU FFN over
    the gathered tokens (slot plan chosen to minimize the static per-core
    token capacity; heavy experts may spill a small slice into a third
    slot on another core); combine weight applied on device; bf16 outputs.
  Host: scatter-add per-expert outputs into the [T, H] result in expert
    order (matches the reference scan accumulation order).

Scale bookkeeping (per-tensor power-of-2 scales, folded into constants):
  x*SX, w1*SW1, w3*SW3, w2*SW2 quantized to fp8 hi+lo.
  PSUM_g = SX*SW1 * g      -> silu input scale 1/(SX*SW1)
  PSUM_u = SX*SW3 * u      -> h' = silu(g) * PSUM_u = SX*SW3 * h
  h' quantized to fp8 hi+lo directly (|h'| < 240 by choice of SW3).
  PSUM_y = SX*SW3*SW2 * y  -> combine weight folded: cvec = c/(SX*SW3*SW2)
"""

import ml_dtypes
import numpy as np

import concourse.bass as bass  # noqa: F401  (engine plumbing)
import concourse.tile as tile
from concourse import bacc, mybir
from concourse.bass_utils import run_bass_kernel_spmd

T, H, F, E, TOPK = 4096, 1024, 512, 16, 4
NCORES = 8
F32 = mybir.dt.float32
BF16 = mybir.dt.bfloat16
FP8 = mybir.dt.float8e4
NPFP8 = ml_dtypes.float8_e4m3

SX, SW1, SW3, SW2 = 2.0, 64.0, 8.0, 64.0
SILU_SCALE = 1.0 / (SX * SW1)          # PSUM_g -> true g
CSCALE = 1.0 / (SX * SW3 * SW2)        # PSUM_y -> true y, folded into cvec

KP = H // 256    # stage-1 contraction k-pairs (DoubleRow: 256 per pair)
FPAIR = F // 256  # stage-2 contraction f-pairs

_nc_cache: dict = {}
LAST_CAPS = (1408, 1024)  # caps used by the most recent kernel() call


def _chunk_list(caps):
    """(slot, t0, tl) chunks of <=512 tokens covering all slots.

    The smallest chunk is moved last so the kernel's drain tail (final
    stage-2 + output DMA) is as short as possible.
    """
    out = []
    t0 = 0
    for s in range(len(caps)):
        rem = caps[s]
        while rem > 0:
            tl = min(512, rem)
            out.append((s, t0, tl))
            t0 += tl
            rem -= tl
    smallest = min(range(len(out)), key=lambda i: out[i][2])
    out.append(out.pop(smallest))
    return out


def _plan_slots(counts):
    """Choose per-core slot capacities + (expert, piece) -> slot assignment.

    Returns (caps, assign) where assign[core][slot] = (expert, lo, hi) token
    sub-range of that expert's token list (or None), minimizing the static
    per-core token capacity sum(caps).

    Plan A (always feasible): slot0 = 8 heaviest experts, slot1 = 8
    lightest; caps = (pad(max heavy), pad(max light)).
    Plan B: two 1024-token base slots + one small overflow slot holding the
    spill of experts that exceed 1024 (at most one spill piece per core).
    """
    E_ = len(counts)
    order = sorted(range(E_), key=lambda e: -counts[e])
    ncores = NCORES

    # Plan A
    cap_a0 = _pad128(max(counts[e] for e in order[:ncores]))
    cap_b = _pad128(max(counts[e] for e in order[ncores:]))
    capsA = (cap_a0, cap_b)
    def piece(e, lo, hi):
        return (e, lo, hi) if hi > lo else None

    assignA = []
    for c in range(ncores):
        e0, e1 = order[c], order[ncores + c]
        assignA.append([piece(e0, 0, counts[e0]), piece(e1, 0, counts[e1])])

    best_caps, best_assign = capsA, assignA

    # Plan B: base slots (base_a, cap_b) + small overflow slot cap_c; the
    # spill of experts exceeding base_a is cut into <=cap_c pieces, one
    # piece per core's overflow slot (<= 8 pieces total).
    for base_a in range(1024, cap_a0, 128):
        for cap_c in (128, 256, 384):
            if base_a + cap_b + cap_c >= sum(best_caps):
                continue
            spills = []
            for e in order[:ncores]:
                rem = counts[e] - base_a
                lo = base_a
                while rem > 0:
                    take = min(cap_c, rem)
                    spills.append((e, lo, lo + take))
                    lo += take
                    rem -= take
            if len(spills) > ncores:
                continue
            assignB = []
            for c in range(ncores):
                e0, e1 = order[c], order[ncores + c]
                row = [piece(e0, 0, min(counts[e0], base_a)),
                       piece(e1, 0, counts[e1]),
                       spills[c] if c < len(spills) else None]
                assignB.append(row)
            best_caps, best_assign = (base_a, cap_b, cap_c), assignB

    # validate: every expert fully covered by in-cap pieces; else Plan A
    covered = [0] * E_
    ok = True
    for row in best_assign:
        for s, piece in enumerate(row):
            if piece is None:
                continue
            e, lo, hi = piece
            ok = ok and 0 < hi - lo <= best_caps[s]
            covered[e] += hi - lo
    if not ok or covered != list(counts):
        best_caps, best_assign = capsA, assignA
    return best_caps, best_assign


def _build_phase_b(caps: tuple):
    """Expert FFN, fp8 DoubleRow with hi/lo error compensation.

    Inputs per core (NS = number of expert slots):
      w13q [NS, 2, H, 2F] per-slot, per-(hi,lo) hstack(w1[e].T*SW1, w3[e].T*SW3)
      w2q  [NS, 2, F, H]  per-slot, per-(hi,lo) w2[e].T*SW2
      xgq  [2, H, CT]     per-(hi,lo) gathered tokens (transposed), fp8
      cvec [128, CT/128]  combine weight * CSCALE per gathered token
    Output:
      yg   [CT, H]        combine-weighted expert outputs, bf16
    """
    DR = mybir.MatmulPerfMode.DoubleRow
    SILU = mybir.ActivationFunctionType.Silu
    COPY = mybir.ActivationFunctionType.Copy
    NS = len(caps)
    CT = sum(caps)
    assert CT % 128 == 0
    nc = bacc.Bacc("TRN2", target_bir_lowering=False, debug=False,
                   num_devices=NCORES)
    w13q = nc.dram_tensor("w13q", [NS, 2, H, 2 * F], FP8,
                          kind="ExternalInput").ap()
    w2q = nc.dram_tensor("w2q", [NS, 2, F, H], FP8, kind="ExternalInput").ap()
    xgq = nc.dram_tensor("xgq", [2, H, CT], FP8, kind="ExternalInput").ap()
    cvec = nc.dram_tensor("cvec", [128, CT // 128], F32,
                          kind="ExternalInput").ap()
    yg = nc.dram_tensor("yg", [CT, H], BF16, kind="ExternalOutput").ap()

    chunks = _chunk_list(caps)

    with tile.TileContext(nc) as tc:
        with (
            tc.tile_pool(name="w13_p", bufs=1) as w13_p,
            tc.tile_pool(name="w2_p", bufs=1) as w2_p,
            tc.tile_pool(name="xg_p", bufs=2) as xg_p,
            tc.tile_pool(name="sg_p", bufs=2) as sg_p,
            tc.tile_pool(name="hp_p", bufs=2) as hp_p,
            tc.tile_pool(name="hq_p", bufs=2) as hq_p,
            tc.tile_pool(name="y_p", bufs=2) as y_p,
            tc.tile_pool(name="c_p", bufs=1) as c_p,
            tc.tile_pool(name="ps", bufs=8, space="PSUM") as ps_pool,
        ):
            c_sb = c_p.tile([128, CT // 128], F32)

            # Weights. w13 hi split per k-pair so the first matmuls only wait
            # on a 256KB DMA; lo and w2 arrive while hi*hi matmuls run.
            whi13 = [[w13_p.tile([128, 2, 2 * F], FP8, name=f"whi13_{s}_{kp}")
                      for kp in range(KP)] for s in range(NS)]
            wlo13 = [[w13_p.tile([128, 2, 2 * F], FP8, name=f"wlo13_{s}_{kp}")
                      for kp in range(KP)] for s in range(NS)]
            whi2 = [w2_p.tile([128, FPAIR, 2, H], FP8, name=f"whi2_{s}")
                    for s in range(NS)]
            wlo2 = [w2_p.tile([128, FPAIR, 2, H], FP8, name=f"wlo2_{s}")
                    for s in range(NS)]

            def load_w13(s, hi):
                tiles, v = (whi13, 0) if hi else (wlo13, 1)
                eng = nc.sync if hi else nc.gpsimd
                for kp in range(KP):
                    eng.dma_start(
                        tiles[s][kp][:],
                        w13q[s, v, kp * 256:(kp + 1) * 256, :].rearrange(
                            "(two p) f -> p two f", p=128))

            def load_w2(s):
                nc.gpsimd.dma_start(
                    whi2[s][:],
                    w2q[s, 0].rearrange("(fp two p) h -> p fp two h", p=128, two=2))
                nc.gpsimd.dma_start(
                    wlo2[s][:],
                    w2q[s, 1].rearrange("(fp two p) h -> p fp two h", p=128, two=2))

            # PE clock warm-up: dummy matmuls on a memset tile while the
            # first weight/activation DMAs stream in (the p-state model
            # upclocks after ~3us of continuous PE activity).
            warm = c_p.tile([128, 2, 128], FP8, name="warm")
            nc.gpsimd.memset(warm[:], 0)
            ps_warm = ps_pool.tile([128, 128], F32, tag="ps", name="ps_warm")
            for _ in range(44):
                nc.tensor.matmul(ps_warm[:], lhsT=warm[:], rhs=warm[:],
                                 start=True, stop=True, perf_mode=DR)



            xgq_r = [xgq[v].rearrange("(kp two p) t -> p kp two t", p=128, two=2)
                     for v in range(2)]

            def stage2(s, t0, tl, hq_hi, hq_lo, last=False):
                y_sb = y_p.tile([128, 4, H], BF16, tag="y",
                                name=f"y_{t0}")
                for tt0 in range(0, tl, 128):
                    cidx = (t0 + tt0) // 128
                    for hh in range(2):
                        ps_y = ps_pool.tile([128, 512], F32, tag="ps",
                                            name=f"psy_{t0}_{tt0}_{hh}")
                        idx = 0
                        for ht, wt in ((hq_hi, whi2[s]), (hq_hi, wlo2[s]),
                                       (hq_lo, whi2[s])):
                            for fp in range(FPAIR):
                                nc.tensor.matmul(
                                    ps_y[:],
                                    lhsT=ht[:, fp, :, tt0:tt0 + 128],
                                    rhs=wt[:, fp, :, hh * 512:(hh + 1) * 512],
                                    start=(idx == 0),
                                    stop=(idx == 3 * FPAIR - 1),
                                    perf_mode=DR)
                                idx += 1
                        if last:
                            # drain tail: per-half tiles (no shared-tile WAW
                            # between the two scale engines), scaled on
                            # ACT/DVE in parallel, shipped from two queues.
                            # The chain-critical hh=1 half goes on the
                            # lower-latency SP queue.
                            yh = y_p.tile([128, 512], BF16, tag=f"yt{hh}",
                                          name=f"ytail_{t0}_{tt0}_{hh}")
                            final = tt0 + 128 >= tl and hh == 1
                            if hh == 1 and not final:
                                nc.vector.tensor_scalar(
                                    yh[:], ps_y[:], c_sb[:, cidx:cidx + 1],
                                    None, op0=mybir.AluOpType.mult)
                            else:
                                # ACT is slightly faster per element; use it
                                # for the chain-critical final piece too
                                nc.scalar.activation(
                                    yh[:], ps_y[:], COPY,
                                    scale=c_sb[:, cidx:cidx + 1])
                            if final:
                                # same-queue issue: the DMA sits right
                                # behind its scale on the ACT queue, no
                                # cross-engine sem hop
                                eng = nc.scalar
                            else:
                                eng = nc.gpsimd if hh == 0 else nc.sync
                            eng.dma_start(
                                yg[t0 + tt0:t0 + tt0 + 128,
                                   hh * 512:(hh + 1) * 512], yh[:])
                            continue
                        ydst = y_sb[:, tt0 // 128, hh * 512:(hh + 1) * 512]
                        nc.scalar.activation(
                            ydst, ps_y[:], COPY,
                            scale=c_sb[:, cidx:cidx + 1])
                    # ship each 128-token tile as soon as it is scaled, so
                    # output traffic overlaps compute instead of piling up
                    # at the drain; alternate queues to halve serialization
                    if not last:
                        eng = nc.sync if ((t0 + tt0) // 128) % 2 else nc.gpsimd
                        eng.dma_start(
                            yg[t0 + tt0:t0 + tt0 + 128], y_sb[:, tt0 // 128])

            pending = None
            for ci, (s, t0, tl) in enumerate(chunks):
                xhi = xg_p.tile([128, KP, 2, 512], FP8, tag="xhi",
                                name=f"xhi_{ci}")
                xlo = xg_p.tile([128, KP, 2, 512], FP8, tag="xlo",
                                name=f"xlo_{ci}")
                if ci == 0:
                    # per-k-pair pieces on two parallel DMA channels: weights
                    # on the SP queue, activations on the scalar queue, so
                    # the k-th hi*hi wave starts after two small transfers.
                    # The very first 128 lhsT columns go in a tiny DMA so
                    # matmul 0 fires as early as possible.
                    nc.sync.dma_start(
                        whi13[s][0][:, :, :128],
                        w13q[s, 0, :256, :128].rearrange(
                            "(two p) f -> p two f", p=128))
                    nc.sync.dma_start(
                        whi13[s][0][:, :, 128:],
                        w13q[s, 0, :256, 128:].rearrange(
                            "(two p) f -> p two f", p=128))
                    for kp in range(1, KP):
                        nc.sync.dma_start(
                            whi13[s][kp][:],
                            w13q[s, 0, kp * 256:(kp + 1) * 256, :].rearrange(
                                "(two p) f -> p two f", p=128))
                    for kp in range(KP):
                        nc.scalar.dma_start(xhi[:, kp, :, :tl],
                                            xgq_r[0][:, kp, :, t0:t0 + tl])
                else:
                    nc.sync.dma_start(xhi[:, :, :, :tl],
                                      xgq_r[0][:, :, :, t0:t0 + tl])
                if ci == 0:
                    # halves on the otherwise-idle scalar queue: the hi*lo
                    # wave then only waits on these, not on wlo13 (waits are
                    # coalesced per queue)
                    nc.scalar.dma_start(xlo[:, :2, :, :tl],
                                        xgq_r[1][:, :2, :, t0:t0 + tl])
                    nc.scalar.dma_start(xlo[:, 2:, :, :tl],
                                        xgq_r[1][:, 2:, :, t0:t0 + tl])
                else:
                    nc.scalar.dma_start(xlo[:, :, :, :tl],
                                        xgq_r[1][:, :, :, t0:t0 + tl])
                if ci == 0:
                    # chunk-0 needs only its own slot's w13 (hi first; the
                    # hi*hi products run while xlo/wlo13 stream in)
                    load_w13(s, hi=False)
                    nc.gpsimd.dma_start(c_sb[:], cvec[:])
                    w13_loaded = {s}
                    w2_loaded = set()
                else:
                    # demand-staged prefetch: next chunk's w13, current and
                    # next slot's w2, each about one chunk ahead of first use
                    nxt = chunks[ci + 1][0] if ci + 1 < len(chunks) else None
                    if s not in w13_loaded:
                        load_w13(s, hi=True)
                        load_w13(s, hi=False)
                        w13_loaded.add(s)
                    if s not in w2_loaded:
                        load_w2(s)
                        w2_loaded.add(s)
                    if nxt is not None and nxt not in w13_loaded:
                        load_w13(nxt, hi=True)
                        load_w13(nxt, hi=False)
                        w13_loaded.add(nxt)
                    if nxt is not None and nxt not in w2_loaded:
                        load_w2(nxt)
                        w2_loaded.add(nxt)

                hq_hi = hq_p.tile([128, FPAIR, 2, 512], FP8, tag="hqhi",
                                  name=f"hqhi_{ci}")
                hq_lo = hq_p.tile([128, FPAIR, 2, 512], FP8, tag="hqlo",
                                  name=f"hqlo_{ci}")
                ps_g = [ps_pool.tile([128, 512], F32, tag="ps",
                                     name=f"psg_{ci}_{fi}") for fi in range(4)]
                ps_u = [ps_pool.tile([128, 512], F32, tag="ps",
                                     name=f"psu_{ci}_{fi}") for fi in range(4)]

                def mm_s1(prod, fi, path, kp, first, last):
                    wt, xt = ((whi13[s], xhi), (whi13[s], xlo),
                              (wlo13[s], xhi))[prod]
                    ps = (ps_g, ps_u)[path][fi]
                    col0 = path * F + fi * 128
                    nc.tensor.matmul(
                        ps[:, :tl], lhsT=wt[kp][:, :, col0:col0 + 128],
                        rhs=xt[:, kp, :, :tl], start=first, stop=last,
                        perf_mode=DR)

                if ci == 0:
                    # hi*hi first across all groups, kp-major (each kp wave
                    # needs only one small weight+activation DMA pair); then
                    # finish each group in turn so PSUM banks free
                    # progressively
                    for kp in range(KP):
                        for fi in range(4):
                            for path in range(2):
                                mm_s1(0, fi, path, kp, kp == 0, False)
                    for fi in range(4):
                        for path in range(2):
                            for prod in (1, 2):
                                for kp in range(KP):
                                    mm_s1(prod, fi, path, kp, False,
                                          prod == 2 and kp == KP - 1)
                else:
                    for fi in range(4):
                        for path in range(2):
                            idx = 0
                            for prod in range(3):
                                for kp in range(KP):
                                    mm_s1(prod, fi, path, kp, idx == 0,
                                          idx == 3 * KP - 1)
                                    idx += 1

                for fi in range(4):
                    fp, two = fi // 2, fi % 2
                    sg = sg_p.tile([128, 512], F32, tag="sg",
                                   name=f"sg_{ci}_{fi}")
                    nc.scalar.activation(sg[:, :tl], ps_g[fi][:, :tl], SILU,
                                         scale=SILU_SCALE)
                    hp = hp_p.tile([128, 512], F32, tag="hp",
                                   name=f"hp_{ci}_{fi}")
                    nc.vector.tensor_mul(hp[:, :tl], sg[:, :tl],
                                         ps_u[fi][:, :tl])
                    nc.vector.tensor_copy(hq_hi[:, fp, two, :tl], hp[:, :tl])
                    nc.vector.tensor_sub(hq_lo[:, fp, two, :tl], hp[:, :tl],
                                         hq_hi[:, fp, two, :tl])

                if pending is not None:
                    stage2(*pending)
                pending = (s, t0, tl, hq_hi, hq_lo)
            stage2(*pending, last=True)

    nc.compile()
    return nc


def _phase_b_nc(caps):
    key = ("b", caps)
    if key not in _nc_cache:
        _nc_cache[key] = _build_phase_b(caps)
    return _nc_cache[key]


def _pad128(n: int) -> int:
    return max(128, (n + 127) // 128 * 128)


def _hilo(a: np.ndarray, scale: float):
    """fp8-e4m3 hi/lo decomposition of a*scale (hi + lo ~= a*scale)."""
    s = (a * scale).astype(np.float32)
    hi = s.astype(NPFP8)
    lo = (s - hi.astype(np.float32)).astype(NPFP8)
    return hi, lo


def kernel(hidden_states, gate_w, bias, w1, w3, w2):
    x = np.ascontiguousarray(np.asarray(hidden_states, dtype=np.float32))
    gate_w = np.asarray(gate_w, dtype=np.float32)
    bias = np.asarray(bias, dtype=np.float32)
    w1 = np.asarray(w1, dtype=np.float32)
    w3 = np.asarray(w3, dtype=np.float32)
    w2 = np.asarray(w2, dtype=np.float32)

    # ----

    # Routing on host, f32 (matches reference math; top-k ties -> lower idx).
    logits = x @ gate_w.T                               # [T, E]
    scores = 1.0 / (1.0 + np.exp(-logits))
    topi = np.argsort(-(scores + bias[None, :]), axis=1,
                      kind="stable")[:, :TOPK]          # [T, K]
    topw = np.take_along_axis(scores, topi, axis=1)
    topw = topw / topw.sum(axis=1, keepdims=True)
    combine = np.zeros((T, E), dtype=np.float32)
    np.put_along_axis(combine, topi, topw, axis=1)      # [T, E]

    # ---- Host dispatch: slot plan (expert pieces -> core slots) ----
    idx_per_e = [np.nonzero(combine[:, e] > 0.0)[0] for e in range(E)]
    counts = [len(ix) for ix in idx_per_e]
    caps, assign = _plan_slots(counts)
    global LAST_CAPS
    LAST_CAPS = caps
    NS = len(caps)
    CT = sum(caps)
    offs = [sum(caps[:s]) for s in range(NS)]

    # ---- Host quantization: fp8 hi/lo of activations and weights ----
    xT = np.ascontiguousarray(x.T)                      # [H, T]
    xhi, xlo = _hilo(xT, SX)

    w13q_per_e = {}
    w2q_per_e = {}
    for e in set(p[0] for row in assign for p in row if p is not None):
        w13 = np.concatenate([w1[e].T * SW1, w3[e].T * SW3], axis=1)
        w13q_per_e[e] = _hilo(w13, 1.0)
        w2q_per_e[e] = _hilo(w2[e].T, SW2)

    in_maps = []
    for c in range(NCORES):
        idx_pad = np.zeros(CT, dtype=np.int64)
        cv = np.zeros(CT, dtype=np.float32)
        w13q = np.zeros((NS, 2, H, 2 * F), dtype=NPFP8)
        w2q = np.zeros((NS, 2, F, H), dtype=NPFP8)
        for s, piece in enumerate(assign[c]):
            if piece is None:
                continue
            e, lo_i, hi_i = piece
            ix = idx_per_e[e][lo_i:hi_i]
            idx_pad[offs[s]:offs[s] + len(ix)] = ix
            cv[offs[s]:offs[s] + len(ix)] = combine[ix, e] * CSCALE
            w13q[s, 0], w13q[s, 1] = w13q_per_e[e]
            w2q[s, 0], w2q[s, 1] = w2q_per_e[e]
        xgq = np.stack([np.ascontiguousarray(xhi[:, idx_pad]),
                        np.ascontiguousarray(xlo[:, idx_pad])])  # [2, H, CT]
        cv_tiled = np.ascontiguousarray(cv.reshape(CT // 128, 128).T)
        in_maps.append({"w13q": w13q, "w2q": w2q, "xgq": xgq,
                        "cvec": cv_tiled})

    # ---- Phase B: expert FFN on device (expert-parallel) ----
    ncB = _phase_b_nc(caps)
    resB = run_bass_kernel_spmd(ncB, in_maps, core_ids=list(range(NCORES)))

    # ---- Host combine: scatter-add in expert order ----
    out = np.zeros((T, H), dtype=np.float32)
    pieces_by_e = [[] for _ in range(E)]
    for c in range(NCORES):
        for s, piece in enumerate(assign[c]):
            if piece is None:
                continue
            e, lo_i, hi_i = piece
            pieces_by_e[e].append((c, offs[s], lo_i, hi_i))
    for e in range(E):
        for c, off, lo_i, hi_i in pieces_by_e[e]:
            ix = idx_per_e[e][lo_i:hi_i]
            if len(ix):
                out[ix] += resB.results[c]["yg"][off:off + len(ix)
                                                 ].astype(np.float32)
    return out
